# revision 1
# baseline (speedup 1.0000x reference)
"""Trainium2 Bass kernel for DynConvLayer (512x512, C=64, K=3, dil=2).

out = where(sd, gelu(conv2(rpad(x_ori))), gelu(dwconv3(rpad(x)))) + x
  x_ori = where(md, gelu(conv1(rpad(x))), x)
  md = 5x5-binary-dilate(mask), sd = mask>0.5, rpad = reflect-pad-2

Sharding: H split across 8 cores (64 rows each, halo 4), W split into 4
panels of 128 cols per core (SBUF capacity). Convs are computed on the
tensor engine as per-tap [C_in, C_out] matmuls accumulated in PSUM over a
flattened (row*136+col) pixel stream; a +2-row-shifted copy of the image
in SBUF partitions 64..127 lets one K=128 matmul cover two taps, and the
depthwise conv3 rides in PSUM partitions 64..127 of the conv1 matmuls as
diagonal weight columns (its gelu lands in the output tile via a
cross-partition ACT). Matmul inputs are bf16 (fp32 PSUM accumulate); the
residual +x is added on the host in fp32. Reflection halos are handled by
host padding plus on-chip strided fix-up copies and per-core edge-mask
blends, keeping the program SPMD-uniform across all 8 cores.
"""

import os
import sys

import numpy as np

for _p in ("/opt/trn_rl_repo", "/opt/pypackages"):
    if _p not in sys.path:
        sys.path.insert(0, _p)

import concourse.bass as bass
import concourse.bacc as bacc
import concourse.mybir as mybir
from concourse.tile import TileContext
from concourse.bass_utils import run_bass_kernel_spmd

F32 = mybir.dt.float32
F32R = mybir.dt.float32r
BF16 = mybir.dt.bfloat16
U8 = mybir.dt.uint8
AF = mybir.ActivationFunctionType

C = 64
H = W = 512
NCORES = 8
RPC = H // NCORES          # 64 output rows per core
NP = 4                     # W panels per core
PCOL = W // NP             # 128 output cols per panel
PW = PCOL + 8              # 136 slab width (cols -4..132 rel panel)
XROWS = 72                 # x slab rows (-4..68 rel core block)
OROWS = RPC + 4            # 68 rows of x_ori / g13 (-2..66)
S13 = OROWS * PW           # 9248 conv13 stream length
S2 = RPC * PW              # 8704 conv2 / output stream length
MX = 2                     # front margin of x tile (negative tap offsets)
MO = 2                     # front margin of x_ori tile
XF = MX + S13 + 4 * PW + 8     # x tile free size
XOF = MO + S2 + 4 * PW + 8     # x_ori tile free size
ROWOFF13 = 2 * PW          # conv13 stream -> x tile row offset
ROWOFF2 = 2 * PW           # conv2 stream -> x_ori row offset
CHUNK = 512

_CACHE = {}


def _chunks(total):
    out = []
    off = 0
    while off < total:
        n = min(CHUNK, total - off)
        out.append((off, n))
        off += n
    return out


def _build_program(act=None):
    act = AF.Gelu if act is None else act
    key = ("nc", str(act))
    if key in _CACHE:
        return _CACHE[key]
    nc = bacc.Bacc("TRN2", target_bir_lowering=False, debug=False)

    x_in = nc.declare_dram_parameter("x_in", [NP, 128, XROWS, PW], BF16, isOutput=False)
    md_in = nc.declare_dram_parameter("md_in", [NP, 64, OROWS, PW], U8, isOutput=False)
    sd_in = nc.declare_dram_parameter("sd_in", [NP, 64, RPC, PW], U8, isOutput=False)
    w13p_in = nc.declare_dram_parameter("w13p", [128, 3 * 128], BF16, isOutput=False)
    w13s_in = nc.declare_dram_parameter("w13s", [64, 3 * 128], BF16, isOutput=False)
    w2p_in = nc.declare_dram_parameter("w2p", [128, 3 * 64], BF16, isOutput=False)
    w2s_in = nc.declare_dram_parameter("w2s", [64, 3 * 64], BF16, isOutput=False)
    b1_in = nc.declare_dram_parameter("b1d", [64, 1], F32, isOutput=False)
    b3_in = nc.declare_dram_parameter("b3d", [64, 1], F32, isOutput=False)
    b2_in = nc.declare_dram_parameter("b2", [64, 1], F32, isOutput=False)
    etop_in = nc.declare_dram_parameter("etopm", [64, PW], U8, isOutput=False)
    ebot_in = nc.declare_dram_parameter("ebotm", [64, PW], U8, isOutput=False)
    out_d = nc.declare_dram_parameter("out", [NP, 64, RPC, PCOL], F32, isOutput=True)

    ch13 = _chunks(S13)
    ch2 = _chunks(S2)

    with TileContext(nc) as tc:
        with (
            tc.tile_pool(name="const", bufs=1) as cpool,
            tc.tile_pool(name="xp", bufs=2) as xpool,
            tc.tile_pool(name="xop", bufs=2) as xopool,
            tc.tile_pool(name="g13p", bufs=1) as gpool,
            tc.tile_pool(name="mp", bufs=1) as mpool,
            tc.tile_pool(name="op", bufs=2) as opool,
            tc.tile_pool(name="g2p", bufs=3) as g2pool,
            tc.tile_pool(name="ps13", bufs=8, space="PSUM") as ps13pool,
        ):
            w13pt = cpool.tile([128, 3 * 128], BF16, name="w13pt")
            w13st = cpool.tile([64, 3 * 128], BF16, name="w13st")
            w2pt = cpool.tile([128, 3 * 64], BF16, name="w2pt")
            w2st = cpool.tile([64, 3 * 64], BF16, name="w2st")
            b1t = cpool.tile([64, 1], F32, name="b1t")
            b3t = cpool.tile([64, 1], F32, name="b3t")
            b2t = cpool.tile([64, 1], F32, name="b2t")
            etopt = cpool.tile([64, PW], U8, name="etopt")
            ebott = cpool.tile([64, PW], U8, name="ebott")
            for t, d in (
                (w13pt, w13p_in), (w13st, w13s_in), (w2pt, w2p_in),
                (w2st, w2s_in), (b1t, b1_in), (b3t, b3_in), (b2t, b2_in),
                (etopt, etop_in), (ebott, ebot_in),
            ):
                nc.sync.dma_start(out=t[:, :], in_=d.ap())

            for p in range(NP):
                xt = xpool.tile([128, XF], BF16, name=f"xt{p}", tag="xt")
                xori = xopool.tile([128, XOF], BF16, name=f"xori{p}", tag="xori")
                g1t = gpool.tile([64, S13], BF16, name=f"g1_{p}", tag="g1")
                mdt = mpool.tile([64, S13], U8, name=f"mdt{p}", tag="mdt")
                sdt = mpool.tile([64, S2], U8, name=f"sdt{p}", tag="sdt")
                outt = opool.tile([64, S2], F32, name=f"outt{p}", tag="outt")

                # margins: read by garbage output positions, never used
                nc.vector.memset(xt[0:128, 0:MX], 0.0)
                nc.vector.memset(xt[0:128, MX + XROWS * PW: XF], 0.0)
                nc.vector.memset(xori[0:128, 0:MO], 0.0)
                nc.vector.memset(xori[0:128, MO + S13: XOF], 0.0)
                nc.vector.memset(xori[64:128, MO + S13 - 2 * PW: MO + S13], 0.0)

                # input DMAs (x in 3 row-bands so compute can start early)
                for r0, r1 in ((0, 10), (10, 24), (24, 48), (48, XROWS)):
                    nc.sync.dma_start(
                        out=xt[0:128, MX + r0 * PW: MX + r1 * PW],
                        in_=x_in.ap()[p, :, r0:r1, :].rearrange("a b c -> a (b c)"),
                    )
                # seed x_ori A-half with x rows (-2..66)
                nc.sync.dma_start(
                    out=xori[0:64, MO: MO + S13],
                    in_=x_in.ap()[p, 0:64, 2:2 + OROWS, :].rearrange("a b c -> a (b c)"),
                )
                nc.sync.dma_start(
                    out=mdt[0:64, 0:S13],
                    in_=md_in.ap()[p].rearrange("a b c -> a (b c)"),
                )
                nc.sync.dma_start(
                    out=sdt[0:64, 0:S2],
                    in_=sd_in.ap()[p].rearrange("a b c -> a (b c)"),
                )

                # ---- conv1 + conv3 fused; per-group epilogue releases
                # fixups/blends/B-copy incrementally so conv2 starts gap-free
                xov = xori[0:64, MO: MO + S13].rearrange("a (r c) -> a r c", c=PW)
                done_prev = 0
                b_prev = 0
                for gi in range(0, len(ch13), 8):
                    grp = ch13[gi: gi + 8]
                    pst = [
                        ps13pool.tile([128, CHUNK], F32, name=f"ps13_{p}_{gi + k}", tag="ps13")
                        for k in range(len(grp))
                    ]
                    for wdx in range(6):
                        if wdx < 3:  # tap pair (-2,dc)+(0,dc), K=128
                            dc = 2 * (wdx - 1)
                            lhs = w13pt[0:128, 128 * wdx: 128 * (wdx + 1)]
                            for k, (o, n) in enumerate(grp):
                                nc.tensor.matmul(
                                    pst[k][0:128, 0:n],
                                    lhs,
                                    xt[0:128, MX + o + dc: MX + o + dc + n],
                                    start=(wdx == 0), stop=(wdx == 5),
                                )
                        else:  # single tap (2,dc), K=64
                            dc = 2 * (wdx - 4)
                            lhs = w13st[0:64, 128 * (wdx - 3): 128 * (wdx - 2)]
                            for k, (o, n) in enumerate(grp):
                                off = MX + o + 4 * PW + dc
                                nc.tensor.matmul(
                                    pst[k][0:128, 0:n],
                                    lhs,
                                    xt[0:64, off: off + n],
                                    start=(wdx == 0), stop=(wdx == 5),
                                )
                    for k, (o, n) in enumerate(grp):
                        nc.scalar.activation(
                            g1t[0:64, o: o + n], pst[k][0:64, 0:n],
                            act, bias=b1t[0:64, 0:1],
                        )
                        # gelu(conv3) goes straight into the output tile
                        # (cross-partition ACT: PSUM parts 64..127 -> 0..63)
                        qa = max(o, ROWOFF13)
                        qb = min(o + n, ROWOFF13 + S2)
                        if qa < qb:
                            nc.scalar.activation(
                                outt[0:64, qa - ROWOFF13: qb - ROWOFF13],
                                pst[k][64:128, qa - o: qb - o],
                                act, bias=b3t[0:64, 0:1],
                            )
                        # x_ori := where(md, gelu1, x) in place
                        nc.vector.copy_predicated(
                            xori[0:64, MO + o: MO + o + n],
                            mdt[0:64, o: o + n],
                            g1t[0:64, o: o + n],
                        )
                    # -- group epilogue over fully-predicated rows
                    o_end = grp[-1][0] + grp[-1][1]
                    last = o_end >= S13
                    done = OROWS if last else o_end // PW
                    if done > done_prev:
                        if p == 0:
                            for dst, src in ((2, 6), (3, 5)):
                                nc.vector.tensor_copy(
                                    xov[:, done_prev:done, dst: dst + 1],
                                    xov[:, done_prev:done, src: src + 1],
                                )
                        if p == NP - 1:
                            for dst, src in ((132, 130), (133, 129)):
                                nc.vector.tensor_copy(
                                    xov[:, done_prev:done, dst: dst + 1],
                                    xov[:, done_prev:done, src: src + 1],
                                )
                    if done_prev < 5 <= done:
                        # top reflect blend (rows -2,-1 <- 2,1), cores 0/7 only
                        for dst, src in ((0, 4), (1, 3)):
                            nc.vector.copy_predicated(
                                xori[0:64, MO + dst * PW: MO + (dst + 1) * PW],
                                etopt[0:64, 0:PW],
                                xori[0:64, MO + src * PW: MO + (src + 1) * PW],
                            )
                    if last:
                        for dst, src in ((OROWS - 2, OROWS - 4), (OROWS - 1, OROWS - 5)):
                            nc.vector.copy_predicated(
                                xori[0:64, MO + dst * PW: MO + (dst + 1) * PW],
                                ebott[0:64, 0:PW],
                                xori[0:64, MO + src * PW: MO + (src + 1) * PW],
                            )
                    # B-half pieces: B row r := x_ori row r+2 (rows 64,65 need
                    # the bottom blend, so they wait for the last group)
                    b_hi = OROWS - 2 if last else min(done - 2, OROWS - 4)
                    if b_hi > b_prev:
                        nc.sync.dma_start(
                            out=xori[64:128, MO + b_prev * PW: MO + b_hi * PW],
                            in_=xori[0:64, MO + (b_prev + 2) * PW: MO + (b_hi + 2) * PW],
                        )
                    b_prev = max(b_prev, b_hi)
                    done_prev = done

                # ---- conv2 on x_ori
                out_prev = 0
                for gi in range(0, len(ch2), 8):
                    grp = ch2[gi: gi + 8]
                    pst = [
                        ps13pool.tile([128, CHUNK], F32, name=f"ps2_{p}_{gi + k}", tag="ps13")
                        for k in range(len(grp))
                    ]
                    for wdx in range(6):
                        if wdx < 3:
                            dc = 2 * (wdx - 1)
                            lhs = w2pt[0:128, 64 * wdx: 64 * (wdx + 1)]
                            for k, (o, n) in enumerate(grp):
                                nc.tensor.matmul(
                                    pst[k][0:64, 0:n],
                                    lhs,
                                    xori[0:128, MO + o + dc: MO + o + dc + n],
                                    start=(wdx == 0), stop=(wdx == 5),
                                )
                        else:
                            dc = 2 * (wdx - 4)
                            lhs = w2st[0:64, 64 * (wdx - 3): 64 * (wdx - 2)]
                            for k, (o, n) in enumerate(grp):
                                off = MO + o + 4 * PW + dc
                                nc.tensor.matmul(
                                    pst[k][0:64, 0:n],
                                    lhs,
                                    xori[0:64, off: off + n],
                                    start=(wdx == 0), stop=(wdx == 5),
                                )
                    for k, (o, n) in enumerate(grp):
                        g2t = g2pool.tile([64, CHUNK], F32, name=f"g2_{p}_{gi + k}", tag="g2")
                        nc.scalar.activation(
                            g2t[0:64, 0:n], pst[k][0:64, 0:n],
                            act, bias=b2t[0:64, 0:1],
                        )
                        nc.vector.copy_predicated(
                            outt[0:64, o: o + n],
                            sdt[0:64, o: o + n],
                            g2t[0:64, 0:n],
                        )
                    o_end2 = grp[-1][0] + grp[-1][1]
                    done2 = RPC if o_end2 >= S2 else o_end2 // PW
                    if done2 > out_prev:
                        nc.sync.dma_start(
                            out=out_d.ap()[p, :, out_prev:done2, :],
                            in_=outt[0:64, 0:S2].rearrange(
                                "a (r c) -> a r c", c=PW)[:, out_prev:done2, 4:132],
                        )
                        out_prev = done2

    nc.compile()
    _CACHE[key] = nc
    return nc


def _pack_weights(w1, w2, w3, b1, b2, b3):
    w13p = np.zeros((128, 3, 128), np.float32)
    w13s = np.zeros((64, 3, 128), np.float32)
    w2p = np.zeros((128, 3, 64), np.float32)
    w2s = np.zeros((64, 3, 64), np.float32)
    di = np.arange(64)
    for k in range(3):
        w13p[0:64, k, 0:64] = w1[:, :, 0, k].T
        w13p[64:128, k, 0:64] = w1[:, :, 1, k].T
        w13p[di, k, 64 + di] = w3[:, 0, 0, k]
        w13p[64 + di, k, 64 + di] = w3[:, 0, 1, k]

        w13s[0:64, k, 0:64] = w1[:, :, 2, k].T
        w13s[di, k, 64 + di] = w3[:, 0, 2, k]
        w2p[0:64, k, :] = w2[:, :, 0, k].T
        w2p[64:128, k, :] = w2[:, :, 1, k].T
        w2s[:, k, :] = w2[:, :, 2, k].T
    b13 = None
    return (
        np.ascontiguousarray(w13p.reshape(128, 384)),
        np.ascontiguousarray(w13s.reshape(64, 384)),
        np.ascontiguousarray(w2p.reshape(128, 192)),
        np.ascontiguousarray(w2s.reshape(64, 192)),
        b1.reshape(64, 1).astype(np.float32),
        b3.reshape(64, 1).astype(np.float32),
        b2.reshape(64, 1).astype(np.float32),
    )


def _dilate5(m):
    # 5x5 binary dilation, SAME/zero-pad semantics (max-pool)
    hh, ww = m.shape
    mp = np.pad(m, 2)
    a = np.maximum.reduce([mp[k: k + hh] for k in range(5)])      # [hh, ww+4]
    return np.maximum.reduce([a[:, k: k + ww] for k in range(5)])  # [hh, ww]


def make_in_maps(x, mask, w1, b1, w2, b2, w3, b3):
    import ml_dtypes
    BF = ml_dtypes.bfloat16
    x = np.asarray(x, np.float32)
    mask = np.asarray(mask, np.float32)

    w13p, w13s, w2p, w2s, b1p, b3p, b2p = _pack_weights(
        np.asarray(w1, np.float32), np.asarray(w2, np.float32),
        np.asarray(w3, np.float32), np.asarray(b1, np.float32),
        np.asarray(b2, np.float32), np.asarray(b3, np.float32))
    w13p = w13p.astype(BF); w13s = w13s.astype(BF)
    w2p = w2p.astype(BF); w2s = w2s.astype(BF)

    xp32 = np.pad(x[0], ((0, 0), (4, 6), (4, 4)), mode="reflect")  # [64,522,520]
    xp = xp32.astype(BF)
    m = mask[0, 0]
    md = (_dilate5(m) > 0.5).astype(np.uint8)
    mdp = np.pad(md, ((2, 2), (4, 4)), mode="edge")   # [516,520]
    sdu = (m > 0.5).astype(np.uint8)
    sdp = np.pad(sdu, ((0, 0), (4, 4)))               # [512,520]

    ones = np.ones((64, PW), np.uint8)
    zeros = np.zeros((64, PW), np.uint8)

    in_maps = []
    for i in range(NCORES):
        r0 = RPC * i
        xc = np.empty((NP, 128, XROWS, PW), BF)
        mdc = np.empty((NP, 64, OROWS, PW), np.uint8)
        sdc = np.empty((NP, 64, RPC, PW), np.uint8)
        for p in range(NP):
            c0 = PCOL * p
            xc[p, 0:64] = xp[:, r0: r0 + XROWS, c0: c0 + PW]
            xc[p, 64:128] = xp[:, r0 + 2: r0 + 2 + XROWS, c0: c0 + PW]
            mdc[p] = np.broadcast_to(
                mdp[r0: r0 + OROWS, c0: c0 + PW], (64, OROWS, PW))
            sdc[p] = np.broadcast_to(
                sdp[r0: r0 + RPC, c0: c0 + PW], (64, RPC, PW))
        in_maps.append({
            "x_in": xc, "md_in": mdc, "sd_in": sdc,
            "w13p": w13p, "w13s": w13s, "w2p": w2p, "w2s": w2s,
            "b1d": b1p, "b3d": b3p, "b2": b2p,
            "etopm": ones if i == 0 else zeros,
            "ebotm": ones if i == NCORES - 1 else zeros,
        })

    return in_maps


def kernel(x, mask, w1, b1, w2, b2, w3, b3):
    nc = _build_program()
    in_maps = make_in_maps(x, mask, w1, b1, w2, b2, w3, b3)
    global _last_in_maps
    _last_in_maps = in_maps
    res = run_bass_kernel_spmd(nc, in_maps, list(range(NCORES)))
    out = np.empty((1, C, H, W), np.float32)
    for i in range(NCORES):
        o = res.results[i]["out"]  # [NP, 64, RPC, PCOL]
        out[0, :, RPC * i: RPC * (i + 1), :] = o.transpose(1, 2, 0, 3).reshape(C, RPC, W)
    out += np.asarray(x, np.float32).reshape(1, C, H, W)
    return out



# revision 2
# speedup vs baseline: 1.4418x; 1.4418x over previous
"""Trainium2 Bass kernel for DynConvLayer (512x512, C=64, K=3, dil=2).

out = where(sd, gelu(conv2(rpad(x_ori))), gelu(dwconv3(rpad(x)))) + x
  x_ori = where(md, gelu(conv1(rpad(x))), x)
  md = 5x5-binary-dilate(mask), sd = mask>0.5, rpad = reflect-pad-2

Sharding: H split across 8 cores (64 rows each, halo 4), W split into 4
panels of 128 cols per core (SBUF capacity). conv1+conv3 run on the
tensor engine in bf16 as per-tap [C_in, C_out] matmuls accumulated in
PSUM over a flattened (row*136+col) pixel stream; a +2-row-shifted copy
of the image in SBUF partitions 64..127 lets one K=128 matmul cover two
taps, and the depthwise conv3 rides in PSUM partitions 64..127 as
diagonal weight columns. conv2 runs in fp8e4m3 DoubleRow mode: x_ori is
kept fp8 in SBUF and a hand-built overlapping [128, 2, N] access
pattern (j-stride = 2 rows) packs all three row-taps of one column
offset into a single K=256 matmul, so conv2 needs 3 matmuls per chunk
instead of 6. w2 is scaled x16 to avoid fp8 subnormals and rescaled in
the gelu ACT (scale=1/16). The output tile and DMA are bf16; the
residual +x is added on the host in fp32. Warm-up matmuls at program
start keep the PE HAM clock-gate from running the first wave cold.
"""

import os
import sys

import numpy as np

for _p in ("/opt/trn_rl_repo", "/opt/pypackages"):
    if _p not in sys.path:
        sys.path.insert(0, _p)

import concourse.bass as bass
import concourse.bacc as bacc
import concourse.mybir as mybir
from concourse.tile import TileContext
from concourse.ap import AP as BassAP
from concourse.bass_utils import run_bass_kernel_spmd

F32 = mybir.dt.float32
BF16 = mybir.dt.bfloat16
FP8 = mybir.dt.float8e4
U8 = mybir.dt.uint8
AF = mybir.ActivationFunctionType

C = 64
H = W = 512
NCORES = 8
RPC = H // NCORES          # 64 output rows per core
NP = 4                     # W panels per core
PCOL = W // NP             # 128 output cols per panel
PW = PCOL + 8              # 136 slab width (cols -4..132 rel panel)
XROWS = 72                 # x slab rows (-4..68 rel core block)
OROWS = RPC + 4            # 68 rows of x_ori / g13 (-2..66)
S13 = OROWS * PW           # 9248 conv13 stream length
S2 = RPC * PW              # 8704 conv2 / output stream length
MX = 2                     # front margin of x tile (negative tap offsets)
MO = 2                     # front margin of x_ori tile
XF = MX + S13 + 4 * PW + 8     # x tile free size
XOF = MO + S2 + 4 * PW + 8     # x_ori tile free size
ROWOFF13 = 2 * PW          # conv13 stream -> x tile row offset
CHUNK = 512
W2SCALE = 16.0
NWARM = 24

_CACHE = {}


def _chunks(total):
    out = []
    off = 0
    while off < total:
        n = min(CHUNK, total - off)
        out.append((off, n))
        off += n
    return out


def _build_program(act=None):
    act = AF.Gelu if act is None else act
    key = ("nc", str(act))
    if key in _CACHE:
        return _CACHE[key]
    nc = bacc.Bacc("TRN2", target_bir_lowering=False, debug=False)

    x_in = nc.declare_dram_parameter("x_in", [NP, 128, XROWS, PW], BF16, isOutput=False)
    xo_in = nc.declare_dram_parameter("xo_in", [NP, 64, OROWS, PW], FP8, isOutput=False)
    md_in = nc.declare_dram_parameter("md_in", [NP, 64, OROWS, PW], U8, isOutput=False)
    sd_in = nc.declare_dram_parameter("sd_in", [NP, 64, RPC, PW], U8, isOutput=False)
    w13p_in = nc.declare_dram_parameter("w13p", [128, 3 * 128], BF16, isOutput=False)
    w13s_in = nc.declare_dram_parameter("w13s", [64, 3 * 128], BF16, isOutput=False)
    w2q_in = nc.declare_dram_parameter("w2q", [128, 3 * 128], FP8, isOutput=False)
    b1_in = nc.declare_dram_parameter("b1d", [64, 1], F32, isOutput=False)
    b3_in = nc.declare_dram_parameter("b3d", [64, 1], F32, isOutput=False)
    b2_in = nc.declare_dram_parameter("b2", [64, 1], F32, isOutput=False)
    etop_in = nc.declare_dram_parameter("etopm", [64, PW], U8, isOutput=False)
    ebot_in = nc.declare_dram_parameter("ebotm", [64, PW], U8, isOutput=False)
    out_d = nc.declare_dram_parameter("out", [NP, 64, RPC, PCOL], BF16, isOutput=True)

    ch13 = _chunks(S13)
    ch2 = _chunks(S2)

    with TileContext(nc) as tc:
        with (
            tc.tile_pool(name="const", bufs=1) as cpool,
            tc.tile_pool(name="xp", bufs=2) as xpool,
            tc.tile_pool(name="xop", bufs=2) as xopool,
            tc.tile_pool(name="g13p", bufs=1) as gpool,
            tc.tile_pool(name="mp", bufs=1) as mpool,
            tc.tile_pool(name="op", bufs=2) as opool,
            tc.tile_pool(name="g2p", bufs=3) as g2pool,
            tc.tile_pool(name="ps13", bufs=8, space="PSUM") as ps13pool,
        ):
            w13pt = cpool.tile([128, 3 * 128], BF16, name="w13pt")
            w13st = cpool.tile([64, 3 * 128], BF16, name="w13st")
            w2qt = cpool.tile([128, 3 * 128], FP8, name="w2qt")
            b1t = cpool.tile([64, 1], F32, name="b1t")
            b3t = cpool.tile([64, 1], F32, name="b3t")
            b2t = cpool.tile([64, 1], F32, name="b2t")
            etopt = cpool.tile([64, PW], U8, name="etopt")
            ebott = cpool.tile([64, PW], U8, name="ebott")
            warm = cpool.tile([128, CHUNK], BF16, name="warm")

            # PE warm-up: serialized dummy matmuls (WAW on one PSUM bank)
            # keep the HAM clock-gate busy while the input DMAs land.
            nc.vector.memset(warm[0:128, 0:CHUNK], 0.0)
            wps = ps13pool.tile([128, CHUNK], F32, name="warmps", tag="ps13")
            for _ in range(NWARM):
                nc.tensor.matmul(
                    wps[0:128, 0:CHUNK],
                    warm[0:128, 0:128],
                    warm[0:128, 0:CHUNK],
                    start=True, stop=True,
                )

            for t, d in (
                (w13pt, w13p_in), (w13st, w13s_in), (w2qt, w2q_in),
                (b1t, b1_in), (b3t, b3_in), (b2t, b2_in),
                (etopt, etop_in), (ebott, ebot_in),
            ):
                nc.sync.dma_start(out=t[:, :], in_=d.ap())

            for p in range(NP):
                xt = xpool.tile([128, XF], BF16, name=f"xt{p}", tag="xt")
                xori = xopool.tile([128, XOF], FP8, name=f"xori{p}", tag="xori")
                g1t = gpool.tile([64, S13], BF16, name=f"g1_{p}", tag="g1")
                mdt = mpool.tile([64, S13], U8, name=f"mdt{p}", tag="mdt")
                sdt = mpool.tile([64, S2], U8, name=f"sdt{p}", tag="sdt")
                outt = opool.tile([64, S2], BF16, name=f"outt{p}", tag="outt")

                # margins: read by garbage output positions, never used
                nc.vector.memset(xt[0:128, 0:MX], 0.0)
                nc.vector.memset(xt[0:128, MX + XROWS * PW: XF], 0.0)
                nc.vector.memset(xori[0:128, 0:MO], 0.0)
                nc.vector.memset(xori[0:128, MO + S13: XOF], 0.0)
                nc.vector.memset(xori[64:128, MO + S13 - 2 * PW: MO + S13], 0.0)

                # input DMAs (x in row-bands so compute can start early)
                for r0, r1 in ((0, 10), (10, 24), (24, 48), (48, XROWS)):
                    nc.sync.dma_start(
                        out=xt[0:128, MX + r0 * PW: MX + r1 * PW],
                        in_=x_in.ap()[p, :, r0:r1, :].rearrange("a b c -> a (b c)"),
                    )
                # seed x_ori A-half with fp8(x) rows (-2..65)
                nc.sync.dma_start(
                    out=xori[0:64, MO: MO + S13],
                    in_=xo_in.ap()[p].rearrange("a b c -> a (b c)"),
                )
                nc.sync.dma_start(
                    out=mdt[0:64, 0:S13],
                    in_=md_in.ap()[p].rearrange("a b c -> a (b c)"),
                )
                nc.sync.dma_start(
                    out=sdt[0:64, 0:S2],
                    in_=sd_in.ap()[p].rearrange("a b c -> a (b c)"),
                )

                # ---- conv1 + conv3 fused; per-group epilogue releases
                # fixups/blends/B-copy incrementally so conv2 starts gap-free
                xov = xori[0:64, MO: MO + S13].rearrange("a (r c) -> a r c", c=PW)
                done_prev = 0
                b_prev = 0
                for gi in range(0, len(ch13), 8):
                    grp = ch13[gi: gi + 8]
                    pst = [
                        ps13pool.tile([128, CHUNK], F32, name=f"ps13_{p}_{gi + k}", tag="ps13")
                        for k in range(len(grp))
                    ]
                    for wdx in range(6):
                        if wdx < 3:  # tap pair (-2,dc)+(0,dc), K=128
                            dc = 2 * (wdx - 1)
                            lhs = w13pt[0:128, 128 * wdx: 128 * (wdx + 1)]
                            for k, (o, n) in enumerate(grp):
                                nc.tensor.matmul(
                                    pst[k][0:128, 0:n],
                                    lhs,
                                    xt[0:128, MX + o + dc: MX + o + dc + n],
                                    start=(wdx == 0), stop=(wdx == 5),
                                )
                        else:  # single tap (2,dc), K=64
                            dc = 2 * (wdx - 4)
                            lhs = w13st[0:64, 128 * (wdx - 3): 128 * (wdx - 2)]
                            for k, (o, n) in enumerate(grp):
                                off = MX + o + 4 * PW + dc
                                nc.tensor.matmul(
                                    pst[k][0:128, 0:n],
                                    lhs,
                                    xt[0:64, off: off + n],
                                    start=(wdx == 0), stop=(wdx == 5),
                                )
                    for k, (o, n) in enumerate(grp):
                        nc.scalar.activation(
                            g1t[0:64, o: o + n], pst[k][0:64, 0:n],
                            act, bias=b1t[0:64, 0:1],
                        )
                        # gelu(conv3) goes straight into the output tile
                        # (cross-partition ACT: PSUM parts 64..127 -> 0..63)
                        qa = max(o, ROWOFF13)
                        qb = min(o + n, ROWOFF13 + S2)
                        if qa < qb:
                            nc.scalar.activation(
                                outt[0:64, qa - ROWOFF13: qb - ROWOFF13],
                                pst[k][64:128, qa - o: qb - o],
                                act, bias=b3t[0:64, 0:1],
                            )
                        # x_ori := where(md, gelu1, x) in place (bf16 -> fp8)
                        nc.vector.copy_predicated(
                            xori[0:64, MO + o: MO + o + n],
                            mdt[0:64, o: o + n],
                            g1t[0:64, o: o + n],
                        )
                    # -- group epilogue over fully-predicated rows
                    o_end = grp[-1][0] + grp[-1][1]
                    last = o_end >= S13
                    done = OROWS if last else o_end // PW
                    if done > done_prev:
                        if p == 0:
                            for dst, src in ((2, 6), (3, 5)):
                                nc.vector.tensor_copy(
                                    xov[:, done_prev:done, dst: dst + 1],
                                    xov[:, done_prev:done, src: src + 1],
                                )
                        if p == NP - 1:
                            for dst, src in ((132, 130), (133, 129)):
                                nc.vector.tensor_copy(
                                    xov[:, done_prev:done, dst: dst + 1],
                                    xov[:, done_prev:done, src: src + 1],
                                )
                    if done_prev < 5 <= done:
                        # top reflect blend (rows -2,-1 <- 2,1), cores 0/7 only
                        for dst, src in ((0, 4), (1, 3)):
                            nc.vector.copy_predicated(
                                xori[0:64, MO + dst * PW: MO + (dst + 1) * PW],
                                etopt[0:64, 0:PW],
                                xori[0:64, MO + src * PW: MO + (src + 1) * PW],
                            )
                    if last:
                        for dst, src in ((OROWS - 2, OROWS - 4), (OROWS - 1, OROWS - 5)):
                            nc.vector.copy_predicated(
                                xori[0:64, MO + dst * PW: MO + (dst + 1) * PW],
                                ebott[0:64, 0:PW],
                                xori[0:64, MO + src * PW: MO + (src + 1) * PW],
                            )
                    # B-half pieces: B row r := x_ori row r+2 (rows 64,65 need
                    # the bottom blend, so they wait for the last group)
                    b_hi = OROWS - 2 if last else min(done - 2, OROWS - 4)
                    if b_hi > b_prev:
                        nc.sync.dma_start(
                            out=xori[64:128, MO + b_prev * PW: MO + b_hi * PW],
                            in_=xori[0:64, MO + (b_prev + 2) * PW: MO + (b_hi + 2) * PW],
                        )
                    b_prev = max(b_prev, b_hi)
                    done_prev = done

                # ---- conv2 on fp8 x_ori: DoubleRow, 3 matmuls per chunk.
                # rhs [128, 2, n]: j=0 -> rows (r-2, r), j=1 -> rows (r, r+2);
                # weights zero the duplicated row-0 block in the upper half.
                xob = xori[0:128, 0:1]
                xo_pstride = xob.ap[0][0]
                out_prev = 0
                for gi in range(0, len(ch2), 8):
                    grp = ch2[gi: gi + 8]
                    pst = [
                        ps13pool.tile([64, CHUNK], F32, name=f"ps2_{p}_{gi + k}", tag="ps13")
                        for k in range(len(grp))
                    ]
                    for wdx in range(3):
                        dc = 2 * (wdx - 1)
                        lhs = w2qt[0:128, 128 * wdx: 128 * (wdx + 1)].rearrange(
                            "p (j m) -> p j m", j=2)
                        for k, (o, n) in enumerate(grp):
                            rhs = BassAP(
                                xob.tensor,
                                xob.offset + MO + o + dc,
                                [[xo_pstride, 128], [2 * PW, 2], [1, n]],
                            )
                            nc.tensor.matmul(
                                pst[k][0:64, 0:n],
                                lhs,
                                rhs,
                                start=(wdx == 0), stop=(wdx == 2),
                                perf_mode=mybir.MatmulPerfMode.DoubleRow,
                            )
                    for k, (o, n) in enumerate(grp):
                        g2t = g2pool.tile([64, CHUNK], BF16, name=f"g2_{p}_{gi + k}", tag="g2")
                        nc.scalar.activation(
                            g2t[0:64, 0:n], pst[k][0:64, 0:n],
                            act, bias=b2t[0:64, 0:1], scale=1.0 / W2SCALE,
                        )
                        nc.vector.copy_predicated(
                            outt[0:64, o: o + n],
                            sdt[0:64, o: o + n],
                            g2t[0:64, 0:n],
                        )
                    o_end2 = grp[-1][0] + grp[-1][1]
                    done2 = RPC if o_end2 >= S2 else o_end2 // PW
                    if done2 > out_prev:
                        nc.sync.dma_start(
                            out=out_d.ap()[p, :, out_prev:done2, :],
                            in_=outt[0:64, 0:S2].rearrange(
                                "a (r c) -> a r c", c=PW)[:, out_prev:done2, 4:132],
                        )
                        out_prev = done2

    nc.compile()
    _CACHE[key] = nc
    return nc


def _pack_weights(w1, w2, w3, b1, b2, b3):
    w13p = np.zeros((128, 3, 128), np.float32)
    w13s = np.zeros((64, 3, 128), np.float32)
    w2q = np.zeros((128, 3, 2, 64), np.float32)
    di = np.arange(64)
    for k in range(3):
        w13p[0:64, k, 0:64] = w1[:, :, 0, k].T
        w13p[64:128, k, 0:64] = w1[:, :, 1, k].T
        w13p[di, k, 64 + di] = w3[:, 0, 0, k]
        w13p[64 + di, k, 64 + di] = w3[:, 0, 1, k]

        w13s[0:64, k, 0:64] = w1[:, :, 2, k].T
        w13s[di, k, 64 + di] = w3[:, 0, 2, k]

        # DoubleRow conv2 weights (x W2SCALE): [K=128, kc, j=2, M=64]
        w2q[0:64, k, 0, :] = W2SCALE * w2[:, :, 0, k].T   # row -2 via A(j=0)
        w2q[0:64, k, 1, :] = W2SCALE * w2[:, :, 1, k].T   # row  0 via A(j=1)
        w2q[64:128, k, 1, :] = W2SCALE * w2[:, :, 2, k].T  # row +2 via B(j=1)
    return (
        np.ascontiguousarray(w13p.reshape(128, 384)),
        np.ascontiguousarray(w13s.reshape(64, 384)),
        np.ascontiguousarray(w2q.reshape(128, 384)),
        b1.reshape(64, 1).astype(np.float32),
        b3.reshape(64, 1).astype(np.float32),
        b2.reshape(64, 1).astype(np.float32),
    )


def _dilate5(m):
    # 5x5 binary dilation, SAME/zero-pad semantics (max-pool)
    hh, ww = m.shape
    mp = np.pad(m, 2)
    a = np.maximum.reduce([mp[k: k + hh] for k in range(5)])      # [hh, ww+4]
    return np.maximum.reduce([a[:, k: k + ww] for k in range(5)])  # [hh, ww]


def make_in_maps(x, mask, w1, b1, w2, b2, w3, b3):
    import ml_dtypes
    BF = ml_dtypes.bfloat16
    E4 = ml_dtypes.float8_e4m3
    x = np.asarray(x, np.float32)
    mask = np.asarray(mask, np.float32)

    w13p, w13s, w2q, b1p, b3p, b2p = _pack_weights(
        np.asarray(w1, np.float32), np.asarray(w2, np.float32),
        np.asarray(w3, np.float32), np.asarray(b1, np.float32),
        np.asarray(b2, np.float32), np.asarray(b3, np.float32))
    w13p = w13p.astype(BF); w13s = w13s.astype(BF)
    w2q = w2q.astype(E4)

    xp32 = np.pad(x[0], ((0, 0), (4, 6), (4, 4)), mode="reflect")  # [64,522,520]
    xp = xp32.astype(BF)
    m = mask[0, 0]
    md = (_dilate5(m) > 0.5).astype(np.uint8)
    mdp = np.pad(md, ((2, 2), (4, 4)), mode="edge")   # [516,520]
    sdu = (m > 0.5).astype(np.uint8)
    sdp = np.pad(sdu, ((0, 0), (4, 4)))               # [512,520]

    ones = np.ones((64, PW), np.uint8)
    zeros = np.zeros((64, PW), np.uint8)

    in_maps = []
    for i in range(NCORES):
        r0 = RPC * i
        xc = np.empty((NP, 128, XROWS, PW), BF)
        xoc = np.empty((NP, 64, OROWS, PW), E4)
        mdc = np.empty((NP, 64, OROWS, PW), np.uint8)
        sdc = np.empty((NP, 64, RPC, PW), np.uint8)
        for p in range(NP):
            c0 = PCOL * p
            xc[p, 0:64] = xp[:, r0: r0 + XROWS, c0: c0 + PW]
            xc[p, 64:128] = xp[:, r0 + 2: r0 + 2 + XROWS, c0: c0 + PW]
            xoc[p] = xp32[:, r0 + 2: r0 + 2 + OROWS, c0: c0 + PW].astype(E4)
            mdc[p] = np.broadcast_to(
                mdp[r0: r0 + OROWS, c0: c0 + PW], (64, OROWS, PW))
            sdc[p] = np.broadcast_to(
                sdp[r0: r0 + RPC, c0: c0 + PW], (64, RPC, PW))
        in_maps.append({
            "x_in": xc, "xo_in": xoc, "md_in": mdc, "sd_in": sdc,
            "w13p": w13p, "w13s": w13s, "w2q": w2q,
            "b1d": b1p, "b3d": b3p, "b2": b2p,
            "etopm": ones if i == 0 else zeros,
            "ebotm": ones if i == NCORES - 1 else zeros,
        })

    return in_maps


def kernel(x, mask, w1, b1, w2, b2, w3, b3):
    nc = _build_program()
    in_maps = make_in_maps(x, mask, w1, b1, w2, b2, w3, b3)
    global _last_in_maps
    _last_in_maps = in_maps
    res = run_bass_kernel_spmd(nc, in_maps, list(range(NCORES)))
    out = np.empty((1, C, H, W), np.float32)
    for i in range(NCORES):
        o = np.asarray(res.results[i]["out"], dtype=np.float32)  # [NP,64,RPC,PCOL]
        out[0, :, RPC * i: RPC * (i + 1), :] = o.transpose(1, 2, 0, 3).reshape(C, RPC, W)
    out += np.asarray(x, np.float32).reshape(1, C, H, W)
    return out
